# revision 7
# baseline (speedup 1.0000x reference)
"""Transformer block (dense_transformer) on 8 TRN2 NeuronCores.

Data-parallel over batch (16 items/core), weights replicated, bf16 matmul
datapath with an fp32 residual stream. All layout transposes run on the DMA
XBAR (blocked dma_start_transpose) so the PE does only matmuls; LayerNorm
affine is folded into the weights host-side; causal masking + softmax
denominators come from one vector multiply + a ones-column in the AV matmul.
Emission is software-pipelined: FFN(g-1) matmuls interleave with the
attention stages of group g so the PE never idles long enough to lose its
warm clock.
"""

import numpy as np

import concourse.bass as bass
import concourse.mybir as mybir
from concourse.tile import TileContext
from concourse.vector_clock import ScopedClock

F32 = mybir.dt.float32
BF16 = mybir.dt.bfloat16
AF = mybir.ActivationFunctionType
AX = mybir.AxisListType
ALU = mybir.AluOpType

B, T, C, H, D = 128, 256, 384, 6, 64
F = 4 * C
NCORES = 8
BL = B // NCORES
P = 128
TT = T // P     # 2 token tiles
CT = C // P     # 3 channel tiles
FT = F // P     # 12 ffn-hidden tiles
LN_EPS = 1e-5
CSCALE = float(C) ** -0.5
IP = 2          # items per group
NG = BL // IP   # groups
W = IP * T      # moving width for feature-major matmuls (512)
NSEG = IP * TT  # ln segments per group


class PatchedTileContext(TileContext):
    """Workaround for this container's walrus: BIR instructions may carry at
    most ONE attached sem wait. Hoist extras into standalone waits."""

    def _hoist_multi_waits(self):
        nc = self.nc
        assert self.sems is not None
        sem_by_num = {s.num: s for s in self.sems.allocated().values()}
        for func in nc.m.functions:
            for blk in func.blocks:
                insts = blk.instructions
                i = 0
                while i < len(insts):
                    inst = insts[i]
                    si = inst.sync_info
                    waits = list(si.on_wait) if (si and si.on_wait) else []
                    if len(waits) <= 1:
                        i += 1
                        continue
                    hoist = waits[1:]
                    for w in hoist:
                        if not (
                            w.sync_type == "semaphore"
                            and w.wait_mode == "sem-ge-imm"
                            and w.id in sem_by_num
                        ):
                            raise RuntimeError(
                                f"cannot hoist waits on {inst.name}: {waits}"
                            )
                    del si.on_wait[1:]
                    engine = nc.engines[inst.engine]
                    new_insts = []
                    for w in hoist:
                        wi = engine.wait_ge(sem_by_num[w.id], w.wait_value)
                        new_insts.append(wi.ins)
                    cur_list = nc.cur_bb.bb.instructions
                    for ni in new_insts:
                        cur_list.remove(ni)
                    insts[i:i] = new_insts
                    i += len(new_insts) + 1

    def _drain_and_barrier(self, tick_clock, wait_clock):
        nc = self.nc
        self._hoist_multi_waits()

        drain_inst = nc.sync.drain()
        wait_clock.add_sem_waits(
            drain_inst.ins, ScopedClock({None: tick_clock.global_clock})
        )
        waits = list(drain_inst.ins.sync_info.on_wait or [])
        if len(waits) > 1:
            drain_inst.ins.sync_info.on_wait.clear()
            assert self.sems is not None
            sem_by_num = {s.num: s for s in self.sems.allocated().values()}
            new_waits = []
            for w in waits:
                assert w.sync_type == "semaphore" and w.wait_mode == "sem-ge-imm", w
                new_waits.append(nc.sync.wait_ge(sem_by_num[w.id], w.wait_value))
            bb = nc.cur_bb.bb
            insts = bb.instructions
            names = [i.name for i in insts]
            di = names.index(drain_inst.ins.name)
            tail = insts[di + 1 : di + 1 + len(new_waits)]
            assert len(tail) == len(new_waits)
            insts[di : di + 1 + len(new_waits)] = tail + [drain_inst.ins]

        nc.all_engine_barrier()
        assert self.sems is not None
        popped = nc._tile_sem_poison_stack.pop()
        assert popped is self._sem_poison
        nc.clear_and_free_semaphores(list(self.sems.allocated().values()))
        nc.all_engine_barrier()


def ts(i, n=P):
    return slice(i * n, (i + 1) * n)


def build_nc():
    nc = bass.Bass()
    x_in = nc.dram_tensor("x", [BL, T, C], F32, kind="ExternalInput")
    wq_in = nc.dram_tensor("wqf", [C, C], BF16, kind="ExternalInput")
    wk_in = nc.dram_tensor("wkf", [C, C], BF16, kind="ExternalInput")
    wv_in = nc.dram_tensor("wvf", [C, C], BF16, kind="ExternalInput")
    wp_in = nc.dram_tensor("wpf", [C, C], BF16, kind="ExternalInput")
    w1_in = nc.dram_tensor("w1f", [C, F], BF16, kind="ExternalInput")
    w2_in = nc.dram_tensor("w2f", [F, C], BF16, kind="ExternalInput")
    gb_in = nc.dram_tensor("gb", [4, C], F32, kind="ExternalInput")
    b1_in = nc.dram_tensor("b1v", [F], F32, kind="ExternalInput")
    m_in = nc.dram_tensor("maskf", [P, 2, 384], BF16, kind="ExternalInput")
    out_t = nc.dram_tensor("out", [BL, T, C], F32, kind="ExternalOutput")

    with PatchedTileContext(nc) as tc:
        with tc.tile_pool(name="consts", bufs=1) as consts:
            wq_s = consts.tile([P, CT, C], BF16, tag="wq")
            nc.sync.dma_start(wq_s[:], wq_in.rearrange("(kt p) m -> p kt m", p=P))
            wk_s = consts.tile([P, CT, C], BF16, tag="wk")
            nc.sync.dma_start(wk_s[:], wk_in.rearrange("(kt p) m -> p kt m", p=P))
            wv_s = consts.tile([P, CT, C], BF16, tag="wv")
            nc.sync.dma_start(wv_s[:], wv_in.rearrange("(kt p) m -> p kt m", p=P))
            wp_s = consts.tile([P, CT, C], BF16, tag="wp")
            nc.sync.dma_start(wp_s[:], wp_in.rearrange("(kt p) m -> p kt m", p=P))
            w1_s = consts.tile([P, CT, F], BF16, tag="w1")
            nc.sync.dma_start(w1_s[:], w1_in.rearrange("(kt p) m -> p kt m", p=P))
            w2_s = consts.tile([P, FT, C], BF16, tag="w2")
            nc.sync.dma_start(w2_s[:], w2_in.rearrange("(kt p) m -> p kt m", p=P))
            gb = consts.tile([P, 4, CT], F32, tag="gb")
            nc.sync.dma_start(gb[:], gb_in.rearrange("g (ct p) -> p g ct", p=P))
            b1c = consts.tile([P, FT], F32, tag="b1c")
            nc.sync.dma_start(b1c[:], b1_in.rearrange("(ft p) -> p ft", p=P))
            maskc = consts.tile([P, 2, 384], BF16, tag="mask")
            nc.sync.dma_start(maskc[:], m_in[:])
            epsc = consts.tile([P, 1], F32, tag="eps")
            nc.gpsimd.memset(epsc[:], LN_EPS)

            bq = gb[:, 0, :]
            bk = gb[:, 1, :]
            bpe = gb[:, 2, :]
            b2e = gb[:, 3, :]

            from contextlib import ExitStack

            with ExitStack() as stack:
                pools = {}
                for nm, bufs, space in [
                    ("xp", 3, None), ("x1p", 2, None), ("op", 2, None),
                    ("xnp", 2, None), ("hctp", 3, None), ("qkp", 2, None),
                    ("vp", 2, None), ("wep", 2, None), ("wnp", 2, None),
                    ("wTp", 4, None), ("asbp", 2, None), ("aTp", 2, None),
                    ("saTp", 2, None), ("sap", 2, None), ("zp", 2, None),
                    ("stp", 4, None), ("scps", 2, "PSUM"),
                    ("attps", 2, "PSUM"), ("bigps", 2, "PSUM"),
                ]:
                    kw = {"space": space} if space else {}
                    pools[nm] = stack.enter_context(
                        tc.tile_pool(name=nm, bufs=bufs, **kw)
                    )
                xp, x1p, op, xnp, hctp, qkp, vp = (
                    pools["xp"], pools["x1p"], pools["op"], pools["xnp"],
                    pools["hctp"], pools["qkp"], pools["vp"],
                )
                wep, wnp, wTp, asbp, aTp, saTp, sap = (
                    pools["wep"], pools["wnp"], pools["wTp"], pools["asbp"],
                    pools["aTp"], pools["saTp"], pools["sap"],
                )
                zp, stp, scps, attps, bigps = (
                    pools["zp"], pools["stp"], pools["scps"],
                    pools["attps"], pools["bigps"],
                )

                def load_x(g):
                    xt = xp.tile([P, IP, TT, C], F32, tag="x")
                    nc.sync.dma_start(
                        xt[:],
                        x_in[g * IP : (g + 1) * IP].rearrange(
                            "i (tt p) c -> p i tt c", p=P
                        ),
                    )
                    return xt

                def ln_block(src, hct_tag):
                    """src [P, IP, TT, C] f32 -> normalized bf16, transposed
                    to feature-major h_ct [P, IP, TT, CT, 128]."""
                    st = stp.tile([P, NSEG, 6], F32, tag="bnst")
                    mv = stp.tile([P, NSEG, 2], F32, tag="bnmv")
                    for i in range(IP):
                        for tt in range(TT):
                            s = i * TT + tt
                            nc.vector.bn_stats(st[:, s, :], src[:, i, tt, :])
                    for s in range(NSEG):
                        nc.vector.bn_aggr(mv[:, s, :], st[:, s, :])
                    lnv = stp.tile([P, NSEG], F32, tag="lnv")
                    nc.scalar.activation(lnv[:], mv[:, :, 1], AF.Ln, bias=epsc[:])
                    rstd = stp.tile([P, NSEG], F32, tag="rstd")
                    nc.scalar.activation(rstd[:], lnv[:], AF.Exp, scale=-0.5)
                    xn = xnp.tile([P, NSEG, C], BF16, tag="xn")
                    for i in range(IP):
                        for tt in range(TT):
                            s = i * TT + tt
                            nc.vector.tensor_scalar(
                                xn[:, s, :], src[:, i, tt, :],
                                mv[:, s, 0:1], rstd[:, s : s + 1],
                                ALU.subtract, ALU.mult,
                            )
                    hct = hctp.tile([P, IP, TT, CT, P], BF16, tag=hct_tag)
                    nc.sync.dma_start_transpose(
                        hct[:].rearrange("p i tt ct c -> p (i tt ct) c"),
                        xn[:].rearrange("p s c -> p (s c)"),
                    )
                    return hct

                def mm_rhs(hct, k):
                    return hct[:, :, :, k, :]

                def qkv_block(g, hct):
                    """QKV matmuls for group g; returns qT, kT, v_aug."""
                    qT = qkp.tile([P, CT, IP, T], BF16, tag="qT")
                    kT = qkp.tile([P, CT, IP, T], BF16, tag="kT")
                    for m in range(CT):
                        psq = bigps.tile([P, IP, T], F32, tag="big")
                        psk = bigps.tile([P, IP, T], F32, tag="big")
                        for k in range(CT):
                            nc.tensor.matmul(
                                psq[:], wq_s[:, k, ts(m)], mm_rhs(hct, k),
                                start=(k == 0), stop=(k == CT - 1),
                            )
                            nc.tensor.matmul(
                                psk[:], wk_s[:, k, ts(m)], mm_rhs(hct, k),
                                start=(k == 0), stop=(k == CT - 1),
                            )
                        nc.scalar.activation(
                            qT[:, m, :, :], psq[:], AF.Identity,
                            bias=bq[:, m : m + 1],
                        )
                        nc.scalar.activation(
                            kT[:, m, :, :], psk[:], AF.Identity,
                            bias=bk[:, m : m + 1],
                        )
                    v_aug = vp.tile([P, IP, TT, H, D + 1], BF16, tag="v")
                    for i in range(IP):
                        nc.gpsimd.memset(v_aug[:, i, :, :, D : D + 1], 1.0)
                    for i in range(IP):
                        for st in range(TT):
                            psv = bigps.tile([P, IP, T], F32, tag="big")
                            pvf = psv[:].rearrange("p a b -> p (a b)")
                            for k in range(CT):
                                nc.tensor.matmul(
                                    pvf[:, 0:C],
                                    hct[:, i, st, k, :], wv_s[:, k, :],
                                    start=(k == 0), stop=(k == CT - 1),
                                )
                            nc.vector.tensor_copy(
                                v_aug[:, i, st, :, 0:D],
                                pvf[:, 0:C].rearrange("p (h d) -> p h d", h=H),
                            )
                    return qT, kT, v_aug

                def sc_chunk(g, qT, kT, i, j):
                    """Score matmuls for item i, head pair j (heads 2j, 2j+1).
                    PSUM layout per head: [tt1 scores 0:256 | tt0 0:128 at
                    256:384 | waste]."""
                    sc = scps.tile([P, 2, 4 * P], F32, tag="sc")
                    for hh in range(2):
                        o = 64 * hh
                        nc.tensor.matmul(
                            sc[:, hh, 0:T],
                            qT[o : o + 64, j, i, ts(1)],
                            kT[o : o + 64, j, i, :],
                            start=True, stop=True,
                            tile_position=(o, 0),
                        )
                        nc.tensor.matmul(
                            sc[:, hh, T : T + P],
                            qT[o : o + 64, j, i, ts(0)],
                            kT[o : o + 64, j, i, 0:P],
                            start=True, stop=True,
                            tile_position=(o, 0),
                        )
                    return sc

                def sc_post(sc, i, j):
                    """exp + mask + blocked transpose for one score chunk."""
                    we = wep.tile([P, 2, 384], BF16, tag="we")
                    nc.scalar.activation(we[:], sc[:, :, 0:384], AF.Exp,
                                         scale=CSCALE)
                    wn = wnp.tile([P, 2, 384], BF16, tag="wn")
                    nc.vector.tensor_tensor(wn[:], we[:], maskc[:], ALU.mult)
                    wT = wTp.tile([P, 6, P], BF16, tag="wT")
                    nc.sync.dma_start_transpose(
                        wT[:], wn[:].rearrange("p a b -> p (a b)")
                    )
                    return wT

                def av_chunk(att0, att1, wT, v_aug, i, j):
                    """AV matmuls for head pair j of item i. wT blocks per
                    hh: [0]=tt1/s0, [1]=tt1/s1, [2]=tt0/s0."""
                    for hh in range(2):
                        h = 2 * j + hh
                        cs = slice(h * (D + 1), (h + 1) * (D + 1))
                        nc.tensor.matmul(
                            att1[:, cs], wT[:, 3 * hh + 0, :],
                            v_aug[:, i, 0, h, :],
                            start=True, stop=False, skip_group_check=True,
                        )
                        nc.tensor.matmul(
                            att1[:, cs], wT[:, 3 * hh + 1, :],
                            v_aug[:, i, 1, h, :],
                            start=False, stop=True, skip_group_check=True,
                        )
                        nc.tensor.matmul(
                            att0[:, cs], wT[:, 3 * hh + 2, :],
                            v_aug[:, i, 0, h, :],
                            start=True, stop=True, skip_group_check=True,
                        )

                def attn_norm(att0, att1, attn_sb, i):
                    """Normalize by the ones-column rowsums; write attn_sb."""
                    a3 = [
                        a[:].rearrange("p (h e) -> p h e", h=H)
                        for a in (att0, att1)
                    ]
                    rec = stp.tile([P, TT, H], F32, tag="rec")
                    for tt, av in enumerate(a3):
                        nc.vector.reciprocal(rec[:, tt, :], av[:, :, D])
                    for tt, av in enumerate(a3):
                        for h in range(H):
                            dst = attn_sb[:, i, tt, h * D : (h + 1) * D]
                            if h % 2 == 0:
                                nc.vector.tensor_scalar_mul(
                                    dst, av[:, h, 0:D], rec[:, tt, h : h + 1]
                                )
                            else:
                                nc.scalar.activation(
                                    dst, av[:, h, 0:D], AF.Identity,
                                    scale=rec[:, tt, h : h + 1],
                                )

                def proj_block(g, attnT):
                    saT = saTp.tile([P, IP, TT, CT, P], BF16, tag="saT")
                    for m in range(CT):
                        psj = bigps.tile([P, IP, T], F32, tag="big")
                        for k in range(CT):
                            nc.tensor.matmul(
                                psj[:], wp_s[:, k, ts(m)], attnT[:, :, :, k, :],
                                start=(k == 0), stop=(k == CT - 1),
                            )
                        nc.scalar.activation(
                            saT[:, :, :, m, :],
                            psj[:].rearrange("p i t -> p (i t)").rearrange(
                                "p (a b) -> p a b", b=P
                            ),
                            AF.Identity,
                            bias=bpe[:, m : m + 1],
                        )
                    return saT

                def tr_back(srcT, tag):
                    """[P, IP, TT, CT, 128] -> token-major [P, IP, TT, C]."""
                    dst = sap.tile([P, IP, TT, C], BF16, tag=tag)
                    nc.sync.dma_start_transpose(
                        dst[:].rearrange("p i tt (ct c) -> p (i tt ct) c", c=P),
                        srcT[:].rearrange("p i tt ct c -> p (i tt ct) c"),
                    )
                    return dst

                def w1_block(g, h2ct, z, m):
                    psz = bigps.tile([P, IP, T], F32, tag="big")
                    for k in range(CT):
                        nc.tensor.matmul(
                            psz[:], w1_s[:, k, ts(m)], h2ct[:, :, :, k, :],
                            start=(k == 0), stop=(k == CT - 1),
                        )
                    if m % 2 == 0:
                        nc.scalar.activation(
                            z[:, m, :, :], psz[:], AF.Relu,
                            bias=b1c[:, m : m + 1],
                        )
                    else:
                        nc.vector.tensor_scalar(
                            z[:, m, :, :], psz[:],
                            b1c[:, m : m + 1], 0.0,
                            ALU.add, ALU.max,
                        )

                def w2_block(g, z, yT, m):
                    psy = bigps.tile([P, IP, T], F32, tag="big")
                    for k in range(FT):
                        nc.tensor.matmul(
                            psy[:], w2_s[:, k, ts(m)], z[:, k, :, :],
                            start=(k == 0), stop=(k == FT - 1),
                        )
                    nc.scalar.activation(
                        yT[:, :, :, m, :],
                        psy[:].rearrange("p i t -> p (i t)").rearrange(
                            "p (a b) -> p a b", b=P
                        ),
                        AF.Identity,
                        bias=b2e[:, m : m + 1],
                    )

                def resid_add(dst, a_f32, b_bf16):
                    for i in range(IP):
                        for tt in range(TT):
                            nc.gpsimd.tensor_tensor(
                                dst[:, i, tt, :], a_f32[:, i, tt, :],
                                b_bf16[:, i, tt, :], ALU.add,
                            )

                def store_out(g, o):
                    nc.sync.dma_start(
                        out_t[g * IP : (g + 1) * IP].rearrange(
                            "i (tt p) c -> p i tt c", p=P
                        ),
                        o[:],
                    )

                # ---------------- pipeline ----------------
                # state per group
                xs = {}
                h1 = {}
                qkv = {}
                attnT_d = {}
                x1_d = {}
                h2 = {}
                z_d = {}

                xs[0] = load_x(0)
                xs[1] = load_x(1)
                h1[0] = ln_block(xs[0][:], "hct")
                qkv[0] = qkv_block(0, h1[0])

                for g in range(NG):
                    gp = g - 1
                    # early: ln1 + x-load for upcoming groups
                    if g + 2 < NG:
                        xs[g + 2] = load_x(g + 2)
                    if g + 1 < NG:
                        h1[g + 1] = ln_block(xs[g + 1][:], "hct")

                    qT, kT, v_aug = qkv[g]
                    if gp >= 0:
                        z_d[gp] = zp.tile([P, FT, IP, T], BF16, tag="z", name="z")

                    def w1s(ms):
                        if gp >= 0:
                            for m in ms:
                                w1_block(gp, h2[gp], z_d[gp], m)

                    attn_sb = asbp.tile([P, IP, TT, C], BF16, tag="asb")
                    atts = {}
                    wTs = {}
                    # item 0 scores/av interleaved with W1(g-1)
                    for i in range(IP):
                        att0 = attps.tile([P, H * (D + 1)], F32, tag="att")
                        att1 = attps.tile([P, H * (D + 1)], F32, tag="att")
                        atts[i] = (att0, att1)
                        sc0 = sc_chunk(g, qT, kT, i, 0)
                        wTs[0] = sc_post(sc0, i, 0)
                        sc1 = sc_chunk(g, qT, kT, i, 1)
                        wTs[1] = sc_post(sc1, i, 1)
                        w1s(range(6 * i, 6 * i + 2))
                        sc2 = sc_chunk(g, qT, kT, i, 2)
                        wTs[2] = sc_post(sc2, i, 2)
                        w1s(range(6 * i + 2, 6 * i + 4))
                        av_chunk(att0, att1, wTs[0], v_aug, i, 0)
                        av_chunk(att0, att1, wTs[1], v_aug, i, 1)
                        w1s(range(6 * i + 4, 6 * i + 6))
                        av_chunk(att0, att1, wTs[2], v_aug, i, 2)
                        attn_norm(att0, att1, attn_sb, i)

                    # blocked transpose attn -> attnT
                    attnT = aTp.tile([P, IP, TT, CT, P], BF16, tag="aT")
                    nc.sync.dma_start_transpose(
                        attnT[:].rearrange("p i tt ct c -> p (i tt ct) c"),
                        attn_sb[:].rearrange("p i tt c -> p (i tt c)"),
                    )
                    attnT_d[g] = attnT

                    # W2(g-1) m0,m1 covers the attnT chain latency
                    if gp >= 0:
                        yT = saTp.tile([P, IP, TT, CT, P], BF16, tag="yT")
                        w2_block(gp, z_d[gp], yT, 0)
                        w2_block(gp, z_d[gp], yT, 1)

                    saT = proj_block(g, attnT)

                    if gp >= 0:
                        w2_block(gp, z_d[gp], yT, 2)
                        y = tr_back(yT, "y")
                        o = op.tile([P, IP, TT, C], F32, tag="o")
                        resid_add(o, x1_d[gp][:], y[:])
                        store_out(gp, o)

                    # residual 1 + ln2 for g
                    sa = tr_back(saT, "sa")
                    x1 = x1p.tile([P, IP, TT, C], F32, tag="x1")
                    for i in range(IP):
                        for tt in range(TT):
                            nc.vector.tensor_tensor(
                                x1[:, i, tt, :], xs[g][:, i, tt, :],
                                sa[:, i, tt, :], ALU.add,
                            )
                    x1_d[g] = x1
                    h2[g] = ln_block(x1[:], "hct")

                    # QKV for g+1
                    if g + 1 < NG:
                        qkv[g + 1] = qkv_block(g + 1, h1[g + 1])

                # epilogue: FFN for last group
                gp = NG - 1
                z_d[gp] = zp.tile([P, FT, IP, T], BF16, tag="z", name="z")
                for m in range(FT):
                    w1_block(gp, h2[gp], z_d[gp], m)
                yT = saTp.tile([P, IP, TT, CT, P], BF16, tag="yT")
                for m in range(CT):
                    w2_block(gp, z_d[gp], yT, m)
                y = tr_back(yT, "y")
                o = op.tile([P, IP, TT, C], F32, tag="o")
                resid_add(o, x1_d[gp][:], y[:])
                store_out(gp, o)
    return nc


_NC_CACHE = None


def _get_nc():
    global _NC_CACHE
    if _NC_CACHE is None:
        _NC_CACHE = build_nc()
    return _NC_CACHE


def _host_mask():
    tri = (np.arange(P)[:, None] >= np.arange(P)[None, :]).astype(np.float32)
    m = np.concatenate([np.ones((P, P), np.float32), tri, tri], axis=1)
    return np.stack([m, m], axis=1)  # [P, 2, 384]


def kernel(x, Wq, Wk, Wv, Wp, bp, W1, b1, W2, b2, g1, be1, g2, be2):
    import ml_dtypes

    bf16 = ml_dtypes.bfloat16
    x = np.ascontiguousarray(np.asarray(x, np.float32))
    WqF = np.asarray(Wq, np.float32).transpose(1, 0, 2).reshape(C, C)
    WkF = np.asarray(Wk, np.float32).transpose(1, 0, 2).reshape(C, C)
    WvF = np.asarray(Wv, np.float32).transpose(1, 0, 2).reshape(C, C)
    WpF = np.asarray(Wp, np.float32)
    W1F = np.asarray(W1, np.float32)
    W2F = np.asarray(W2, np.float32)
    g1 = np.asarray(g1, np.float32)
    be1 = np.asarray(be1, np.float32)
    g2 = np.asarray(g2, np.float32)
    be2 = np.asarray(be2, np.float32)
    bp = np.asarray(bp, np.float32)
    b1 = np.asarray(b1, np.float32)
    b2 = np.asarray(b2, np.float32)

    # fold LN affine into the weights
    Wq_e = (g1[:, None] * WqF).astype(bf16)
    Wk_e = (g1[:, None] * WkF).astype(bf16)
    Wv_e = (g1[:, None] * WvF).astype(bf16)
    W1_e = (g2[:, None] * W1F).astype(bf16)
    bq = be1 @ WqF
    bk = be1 @ WkF
    bv = be1 @ WvF
    bp_e = bp + bv @ WpF
    b1_e = b1 + be2 @ W1F
    gbias = np.ascontiguousarray(np.stack([bq, bk, bp_e, b2]))

    nc = _get_nc()
    shared = {
        "wqf": np.ascontiguousarray(Wq_e),
        "wkf": np.ascontiguousarray(Wk_e),
        "wvf": np.ascontiguousarray(Wv_e),
        "wpf": np.ascontiguousarray(WpF.astype(bf16)),
        "w1f": np.ascontiguousarray(W1_e),
        "w2f": np.ascontiguousarray(W2F.astype(bf16)),
        "gb": gbias,
        "b1v": np.ascontiguousarray(b1_e),
        "maskf": np.ascontiguousarray(_host_mask().astype(bf16)),
    }
    in_maps = []
    for c in range(NCORES):
        m = dict(shared)
        m["x"] = np.ascontiguousarray(x[c * BL : (c + 1) * BL])
        in_maps.append(m)

    from concourse.bass_utils import run_bass_kernel_spmd

    res = run_bass_kernel_spmd(nc, in_maps, list(range(NCORES)))
    out = np.concatenate([res.results[c]["out"] for c in range(NCORES)], axis=0)
    return out.astype(np.float32)
